# revision 31
# baseline (speedup 1.0000x reference)
"""Trainium2 Bass kernel for DeformableConv2d (B,H,W,C=8,64,64,128; F=128; 3x3).

Data-parallel over batch: one batch element per NeuronCore (8 cores).

v2 design notes (vs v1 which used 288 per-tile indirect DMAs):
  - Gathers batched via gpsimd dma_gather: one instruction per (kernel point,
    block of 8 pixel tiles) = 36 gathers x 1024 descriptors of 1KB. SWDGE
    fixed cost (~1us/instruction) amortized 8x.
  - xquad DRAM layout: row q = [x[q], x[q+64], x[q+1], x[q+65]] so one 1KB
    contiguous read fetches all 4 bilinear corners for integer corner
    q = y0*64 + x0.
  - floor via round(y - 0.5): grid constant is pre-shifted by -0.5 on host and
    clipped to [-0.499, 62.499]; round-to-nearest of that equals floor(y)
    (or floor(y)-1 with frac weight exactly 1 at integer y - same lerp value).
    No floor/ceil correction ops, and q stays in [0, 4030] so no padding rows.
  - dma_gather wants indices as int16 in a [128, n/16] tile: index i at
    [i%16, i//16], replicated across the 8 gpsimd-core partition groups. A
    second tiny coordinate pipeline on partitions 0..15 computes q in exactly
    that layout from a host-permuted copy of the offsets; 8 small SBUF->SBUF
    DMAs replicate it.
  - Bilinear combine on DVE in pixel-major layout, batched per (kn, block):
    one [128,8,4,128] mult against broadcast corner weights + 3 adds.
  - PE transposes combined tiles to channel-major, matmuls accumulate over
    the 9 kernel points in PSUM, bias via activation, PE transposes back.
"""

import os
from contextlib import ExitStack

import numpy as np

import concourse.bass as bass
import concourse.mybir as mybir
import concourse.tile as tile
from concourse import bacc
from concourse._compat import with_exitstack
from concourse.bass_utils import run_bass_kernel_spmd
from concourse.masks import make_identity

KH, KW, KN = 3, 3, 9
H = W_IMG = 64
C = 128
F = 128
P = 128
NPIX = H * W_IMG            # 4096 pixels per core
NT = NPIX // P              # 32 pixel tiles
NB = 4                      # blocks of 8 tiles
TB = NT // NB               # 8 tiles per block
NIDX = TB * P               # 1024 gathered pixels per dma_gather
ES = 4 * C                  # 512 elems (1KB bf16) per gather descriptor

f32 = mybir.dt.float32
bf16 = mybir.dt.bfloat16
i32 = mybir.dt.int32
i16 = mybir.dt.int16
ALU = mybir.AluOpType
ACT = mybir.ActivationFunctionType

CLIP_LO = -0.499
CLIP_HI = 62.499


def _grid_full():
    """reference._grid_offset flattened to [4096, 18] (y,x interleaved)."""
    init = np.stack(np.meshgrid(np.arange(KH), np.arange(KW), indexing="ij"))
    init = init.reshape(-1, 2).astype(np.float32)
    ph, pw = (KH - 1) // 2, (KW - 1) // 2
    g = np.stack(
        np.meshgrid(np.arange(-ph, H - ph), np.arange(-pw, W_IMG - pw), indexing="ij"),
        axis=-1,
    ).astype(np.float32)
    return (g[:, :, None, :] + init[None, None]).reshape(NPIX, 2 * KN)


@with_exitstack
def _body(ctx: ExitStack, tc: "tile.TileContext", t_off, t_grid, t_off2,
          t_grid2, t_w, t_b, t_out, t_xq):
    nc = tc.nc
    off_ap = t_off.ap()
    grid_ap = t_grid.ap()
    off2_ap = t_off2.ap()
    grid2_ap = t_grid2.ap()
    w_ap = t_w.ap()
    b_ap = t_b.ap()
    out_ap = t_out.ap()
    xq_ap = t_xq.ap()

    const = ctx.enter_context(tc.tile_pool(name="const", bufs=1))
    idxp = ctx.enter_context(tc.tile_pool(name="idx", bufs=1))
    gpool = ctx.enter_context(tc.tile_pool(name="gath", bufs=4))
    qpool = ctx.enter_context(tc.tile_pool(name="qside", bufs=1))
    mpool = ctx.enter_context(tc.tile_pool(name="m", bufs=2))
    abpool = ctx.enter_context(tc.tile_pool(name="ab", bufs=4))
    dpool = ctx.enter_context(tc.tile_pool(name="d", bufs=3))
    dTpool = ctx.enter_context(tc.tile_pool(name="dT", bufs=3))
    oTpool = ctx.enter_context(tc.tile_pool(name="oT", bufs=4))
    opool = ctx.enter_context(tc.tile_pool(name="o", bufs=4))
    ps_dT = ctx.enter_context(tc.tile_pool(name="ps_dT", bufs=2, space="PSUM"))
    ps_out = ctx.enter_context(tc.tile_pool(name="ps_out", bufs=2, space="PSUM"))
    ps_o = ctx.enter_context(tc.tile_pool(name="ps_o", bufs=2, space="PSUM"))

    # ---- dummy gather: starts the gpsimd gather-library load immediately ----
    dummy_idx = const.tile([P, 1], i16)
    nc.gpsimd.memset(dummy_idx[:], 0)
    dummy_out = const.tile([P, 1, ES], bf16)
    nc.gpsimd.dma_gather(dummy_out[:], xq_ap, dummy_idx[:], 16, 16, ES)

    # ---- loads ----
    off2 = qpool.tile([16, KN * 2 * NT * 8], f32)
    nc.scalar.dma_start(off2[:], off2_ap)
    grid2 = qpool.tile([16, KN * 2 * NT * 8], f32)
    nc.scalar.dma_start(grid2[:], grid2_ap)
    offs = idxp.tile([P, NT, 2 * KN], f32)
    nc.scalar.dma_start(offs[:], off_ap.rearrange("p (t k) -> p t k", k=2 * KN))
    grid = idxp.tile([P, NT, 2 * KN], f32)
    nc.scalar.dma_start(grid[:], grid_ap.rearrange("p (t k) -> p t k", k=2 * KN))

    # ---- gather-index pipeline on partitions 0..15, split per block so the
    # first preps can launch after ~1/4 of it ----
    # layout [a, kn, coord, tau, phi]: pixel (p = phi*16+a, tau)
    co2 = qpool.tile([16, KN * 2 * NT * 8], f32)
    ci2 = qpool.tile([16, KN * 2 * NT * 8], i32)
    q32 = qpool.tile([16, KN, NT, 8], i32)
    qw16 = qpool.tile([16, KN, NT, 8], i16)
    qw = idxp.tile([P, KN, NT, 8], i16)

    nc.vector.tensor_add(co2[:], off2[:], grid2[:])
    nc.vector.tensor_scalar(co2[:], co2[:], CLIP_LO, CLIP_HI, ALU.max, ALU.min)
    nc.vector.tensor_copy(ci2[:], co2[:])
    ci2v = ci2[:].rearrange("a (k c t h) -> a k c t h", k=KN, c=2, t=NT)
    nc.vector.tensor_scalar(q32[:], ci2v[:, :, 0, :, :], 6, None,
                            ALU.arith_shift_left)
    nc.vector.tensor_tensor(qw16[:], q32[:], ci2v[:, :, 1, :, :], ALU.add)
    for g in range(8):
        eng = nc.sync if g % 2 == 0 else nc.scalar
        eng.dma_start(qw[g * 16:(g + 1) * 16, :, :, :], qw16[:])

    # ---- corner-weight pipeline (pixel px = p*32 + tau at [p, tau]) ----
    co = idxp.tile([P, NT, 2 * KN], f32)
    nc.vector.tensor_add(co[:], offs[:], grid[:])
    nc.vector.tensor_scalar(co[:], co[:], CLIP_LO, CLIP_HI, ALU.max, ALU.min)
    ci = idxp.tile([P, NT, 2 * KN], i32)
    nc.vector.tensor_copy(ci[:], co[:])            # round-to-nearest
    cf = idxp.tile([P, NT, 2 * KN], f32)
    nc.vector.tensor_copy(cf[:], ci[:])
    fr = idxp.tile([P, NT, 2 * KN], f32)
    nc.vector.tensor_sub(fr[:], co[:], cf[:])      # in [-0.5, 0.5]
    fp = idxp.tile([P, NT, 2 * KN], f32)           # frac = fr + 0.5
    nc.vector.tensor_scalar_add(fp[:], fr[:], 0.5)
    un = idxp.tile([P, NT, 2 * KN], f32)           # 1 - frac = 0.5 - fr
    nc.vector.tensor_scalar(un[:], fr[:], -1.0, 0.5, ALU.mult, ALU.add)

    fpv = fp[:].rearrange("p t (n two) -> p t n two", two=2)
    unv = un[:].rearrange("p t (n two) -> p t n two", two=2)

    # corner weights, order [00, 10, 01, 11] matching xquad layout
    w4 = idxp.tile([P, NT, KN, 4], bf16)
    nc.vector.tensor_tensor(w4[:, :, :, 0], unv[:, :, :, 0], unv[:, :, :, 1], ALU.mult)
    nc.vector.tensor_tensor(w4[:, :, :, 1], fpv[:, :, :, 0], unv[:, :, :, 1], ALU.mult)
    nc.vector.tensor_tensor(w4[:, :, :, 2], unv[:, :, :, 0], fpv[:, :, :, 1], ALU.mult)
    nc.vector.tensor_tensor(w4[:, :, :, 3], fpv[:, :, :, 0], fpv[:, :, :, 1], ALU.mult)



    # ---- constants ----
    ident = const.tile([P, P], f32)
    make_identity(nc, ident[:])
    ident16 = const.tile([P, P], bf16)
    nc.vector.tensor_copy(ident16[:], ident[:])
    w_sb = const.tile([P, KN, F], bf16)
    nc.sync.dma_start(w_sb[:], w_ap)  # [C, KN, F] bf16
    b_sb = const.tile([P, 1], f32)
    nc.sync.dma_start(b_sb[:], b_ap[:, None])

    # ---- main loop: one 1024-descriptor dma_gather per (kn, block). The
    # gather ucode streams descriptors to the DMA engines as it generates
    # them, so the instruction is descriptor-gen-bound (~8.7us/1024). ----
    def prep_gather(out_ap_, idx_ap_, qnum):
        nc.gpsimd.dma_gather(out_ap_, xq_ap, idx_ap_, NIDX, NIDX, ES,
                             queue_num=qnum)

    gidx = 0
    for b in range(NB):
        ops0 = ps_out.tile([P, 512], f32)
        ops1 = ps_out.tile([P, 512], f32)
        for kn in range(KN):
            G = gpool.tile([P, TB, ES], bf16)
            prep_gather(G[:], qw[:, kn, b * TB:(b + 1) * TB, :], gidx % 4)
            gidx += 1
            # xquad element order per row: (jp, c, jq); w4 j = jp*2 + jq.
            # DVE APs max out at 3 free dims, so one mult per jp.
            m = mpool.tile([P, TB, 2, C, 2], bf16)
            G5 = G[:].rearrange("p t (jp c jq) -> p t jp c jq", jp=2, c=C)
            for jp in range(2):
                nc.vector.tensor_tensor(
                    m[:, :, jp, :, :],
                    G5[:, :, jp, :, :],
                    w4[:, b * TB:(b + 1) * TB, kn, 2 * jp:2 * jp + 2]
                        .unsqueeze(2).broadcast_to([P, TB, C, 2]),
                    ALU.mult,
                )
            aa = abpool.tile([P, TB, C, 2], bf16)
            nc.vector.tensor_tensor(aa[:], m[:, :, 0, :, :], m[:, :, 1, :, :],
                                    ALU.add)
            dd = dpool.tile([P, TB, C], bf16)
            nc.vector.tensor_tensor(dd[:], aa[:, :, :, 0], aa[:, :, :, 1],
                                    ALU.add)

            dps = ps_dT.tile([P, TB, P], bf16)
            for tl in range(TB):
                nc.tensor.transpose(dps[:, tl, :], dd[:, tl, :], ident16[:])
            dT = dTpool.tile([P, TB, P], bf16)
            nc.scalar.copy(dT[:], dps[:])
            nc.tensor.matmul(
                ops0[:], lhsT=w_sb[:, kn, :],
                rhs=dT[:, 0:4, :].rearrange("p t c -> p (t c)"),
                start=(kn == 0), stop=(kn == KN - 1),
            )
            nc.tensor.matmul(
                ops1[:], lhsT=w_sb[:, kn, :],
                rhs=dT[:, 4:8, :].rearrange("p t c -> p (t c)"),
                start=(kn == 0), stop=(kn == KN - 1),
            )
        for half, opsx in ((0, ops0), (1, ops1)):
            oT = oTpool.tile([P, 512], f32)
            nc.scalar.activation(oT[:], opsx[:], ACT.Identity,
                                 bias=b_sb[:, 0:1], scale=1.0)
            for t4 in range(4):
                o_ps = ps_o.tile([P, P], f32)
                nc.tensor.transpose(o_ps[:], oT[:, t4 * P:(t4 + 1) * P], ident[:])
                o_sb = opool.tile([P, P], f32)
                nc.scalar.copy(o_sb[:], o_ps[:])
                pix0 = (b * TB + half * 4 + t4) * P
                nc.sync.dma_start(out_ap[pix0:pix0 + P, :], o_sb[:])


def build_nc():
    nc = bacc.Bacc(
        "TRN2",
        target_bir_lowering=False,
        debug=False,
        enable_asserts=False,
        num_devices=8,
        num_swdge_queues=4,
    )
    t_off = nc.dram_tensor("off", [P, NT * 2 * KN], f32, kind="ExternalInput")
    t_grid = nc.dram_tensor("grid", [P, NT * 2 * KN], f32, kind="ExternalInput")
    t_off2 = nc.dram_tensor("off2", [16, KN * 2 * NT * 8], f32, kind="ExternalInput")
    t_grid2 = nc.dram_tensor("grid2", [16, KN * 2 * NT * 8], f32, kind="ExternalInput")
    t_w = nc.dram_tensor("w", [C, KN, F], bf16, kind="ExternalInput")
    t_b = nc.dram_tensor("b", [F], f32, kind="ExternalInput")
    t_out = nc.dram_tensor("out", [NPIX, F], f32, kind="ExternalOutput")
    t_xq = nc.dram_tensor("xquad", [NPIX, ES], bf16, kind="ExternalInput")
    with tile.TileContext(nc) as tc:
        _body(tc, t_off, t_grid, t_off2, t_grid2, t_w, t_b, t_out, t_xq)
    nc.compile()
    return nc


def make_in_maps(x, offset, W, b):
    import ml_dtypes

    B = x.shape[0]
    # pixel relabel: pixel px = p*32 + tau lives at (partition p, tile tau)
    grid_full = _grid_full()                       # [4096, 18]
    grid_host = np.ascontiguousarray(
        grid_full.reshape(P, NT, 2 * KN) - 0.5
    ).reshape(P, NT * 2 * KN).astype(np.float32)
    # [a, kn, coord, tau, phi] <- full[(phi*16+a)*32 + tau, 2kn+coord]
    g5 = (grid_full.reshape(8, 16, NT, KN, 2).transpose(1, 3, 4, 2, 0) - 0.5)
    grid2_host = np.ascontiguousarray(g5.reshape(16, KN * 2 * NT * 8)).astype(np.float32)

    w_host = np.ascontiguousarray(
        np.asarray(W, np.float32).transpose(1, 0, 2).astype(ml_dtypes.bfloat16))
    b_host = np.ascontiguousarray(np.asarray(b, np.float32))

    in_maps = []
    for i in range(B):
        xi = np.asarray(x[i], np.float32).reshape(NPIX, C).astype(ml_dtypes.bfloat16)
        # [q, jp(x-corner), c, jq(y-corner)] = xi[q + 64*jq + jp, c]
        xq = np.zeros((NPIX, 2, C, 2), ml_dtypes.bfloat16)
        xq[:, 0, :, 0] = xi
        xq[:-64, 0, :, 1] = xi[64:]
        xq[:-1, 1, :, 0] = xi[1:]
        xq[:-65, 1, :, 1] = xi[65:]
        off_i = np.asarray(offset[i], np.float32).reshape(NPIX, 2 * KN)
        off2_i = np.ascontiguousarray(
            off_i.reshape(8, 16, NT, KN, 2).transpose(1, 3, 4, 2, 0)
            .reshape(16, KN * 2 * NT * 8)
        )
        in_maps.append(
            {
                "xquad": np.ascontiguousarray(xq.reshape(NPIX, ES)),
                "off": np.ascontiguousarray(off_i.reshape(P, NT * 2 * KN)),
                "off2": off2_i,
                "grid": grid_host,
                "grid2": grid2_host,
                "w": w_host,
                "b": b_host,
            }
        )
    return in_maps


_RESULTS_CACHE = {}


def kernel(x, offset, W, b, _trace=False):
    x = np.asarray(x)
    B = x.shape[0]
    assert x.shape == (B, H, W_IMG, C), x.shape
    nc = build_nc()
    in_maps = make_in_maps(x, offset, W, b)
    res = run_bass_kernel_spmd(nc, in_maps, core_ids=list(range(B)), trace=_trace)
    _RESULTS_CACHE["last"] = res
    out = np.stack(
        [res.results[i]["out"].reshape(NT, P, F).transpose(1, 0, 2)
         .reshape(H, W_IMG, F) for i in range(B)]
    ).astype(np.float32)
    return out


# revision 32
# speedup vs baseline: 1.0313x; 1.0313x over previous
"""Trainium2 Bass kernel for DeformableConv2d (B,H,W,C=8,64,64,128; F=128; 3x3).

Data-parallel over batch: one batch element per NeuronCore (8 cores).

v2 design notes (vs v1 which used 288 per-tile indirect DMAs):
  - Gathers batched via gpsimd dma_gather: one instruction per (kernel point,
    block of 8 pixel tiles) = 36 gathers x 1024 descriptors of 1KB. SWDGE
    fixed cost (~1us/instruction) amortized 8x.
  - xquad DRAM layout: row q = [x[q], x[q+64], x[q+1], x[q+65]] so one 1KB
    contiguous read fetches all 4 bilinear corners for integer corner
    q = y0*64 + x0.
  - floor via round(y - 0.5): grid constant is pre-shifted by -0.5 on host and
    clipped to [-0.499, 62.499]; round-to-nearest of that equals floor(y)
    (or floor(y)-1 with frac weight exactly 1 at integer y - same lerp value).
    No floor/ceil correction ops, and q stays in [0, 4030] so no padding rows.
  - dma_gather wants indices as int16 in a [128, n/16] tile: index i at
    [i%16, i//16], replicated across the 8 gpsimd-core partition groups. A
    second tiny coordinate pipeline on partitions 0..15 computes q in exactly
    that layout from a host-permuted copy of the offsets; 8 small SBUF->SBUF
    DMAs replicate it.
  - Bilinear combine on DVE in pixel-major layout, batched per (kn, block):
    one [128,8,4,128] mult against broadcast corner weights + 3 adds.
  - PE transposes combined tiles to channel-major, matmuls accumulate over
    the 9 kernel points in PSUM, bias via activation, PE transposes back.
"""

import os
from contextlib import ExitStack

import numpy as np

import concourse.bass as bass
import concourse.mybir as mybir
import concourse.tile as tile
from concourse import bacc
from concourse._compat import with_exitstack
from concourse.bass_utils import run_bass_kernel_spmd
from concourse.masks import make_identity

KH, KW, KN = 3, 3, 9
H = W_IMG = 64
C = 128
F = 128
P = 128
NPIX = H * W_IMG            # 4096 pixels per core
NT = NPIX // P              # 32 pixel tiles
NB = 4                      # blocks of 8 tiles
TB = NT // NB               # 8 tiles per block
NIDX = TB * P               # 1024 gathered pixels per dma_gather
ES = 4 * C                  # 512 elems (1KB bf16) per gather descriptor

f32 = mybir.dt.float32
bf16 = mybir.dt.bfloat16
i32 = mybir.dt.int32
i16 = mybir.dt.int16
ALU = mybir.AluOpType
ACT = mybir.ActivationFunctionType

CLIP_LO = -0.499
CLIP_HI = 62.499


def _grid_full():
    """reference._grid_offset flattened to [4096, 18] (y,x interleaved)."""
    init = np.stack(np.meshgrid(np.arange(KH), np.arange(KW), indexing="ij"))
    init = init.reshape(-1, 2).astype(np.float32)
    ph, pw = (KH - 1) // 2, (KW - 1) // 2
    g = np.stack(
        np.meshgrid(np.arange(-ph, H - ph), np.arange(-pw, W_IMG - pw), indexing="ij"),
        axis=-1,
    ).astype(np.float32)
    return (g[:, :, None, :] + init[None, None]).reshape(NPIX, 2 * KN)


@with_exitstack
def _body(ctx: ExitStack, tc: "tile.TileContext", t_off, t_grid, t_off2,
          t_grid2, t_w, t_b, t_out, t_xq):
    nc = tc.nc
    off_ap = t_off.ap()
    grid_ap = t_grid.ap()
    off2_ap = t_off2.ap()
    grid2_ap = t_grid2.ap()
    w_ap = t_w.ap()
    b_ap = t_b.ap()
    out_ap = t_out.ap()
    xq_ap = t_xq.ap()

    const = ctx.enter_context(tc.tile_pool(name="const", bufs=1))
    idxp = ctx.enter_context(tc.tile_pool(name="idx", bufs=1))
    gpool = ctx.enter_context(tc.tile_pool(name="gath", bufs=4))
    qpool = ctx.enter_context(tc.tile_pool(name="qside", bufs=1))
    mpool = ctx.enter_context(tc.tile_pool(name="m", bufs=2))
    abpool = ctx.enter_context(tc.tile_pool(name="ab", bufs=4))
    dpool = ctx.enter_context(tc.tile_pool(name="d", bufs=3))
    dTpool = ctx.enter_context(tc.tile_pool(name="dT", bufs=3))
    oTpool = ctx.enter_context(tc.tile_pool(name="oT", bufs=4))
    opool = ctx.enter_context(tc.tile_pool(name="o", bufs=4))
    ps_dT = ctx.enter_context(tc.tile_pool(name="ps_dT", bufs=2, space="PSUM"))
    ps_out = ctx.enter_context(tc.tile_pool(name="ps_out", bufs=2, space="PSUM"))
    ps_o = ctx.enter_context(tc.tile_pool(name="ps_o", bufs=2, space="PSUM"))

    # ---- dummy gather: starts the gpsimd gather-library load immediately ----
    dummy_idx = const.tile([P, 1], i16)
    nc.gpsimd.memset(dummy_idx[:], 0)
    dummy_out = const.tile([P, 1, ES], bf16)
    nc.gpsimd.dma_gather(dummy_out[:], xq_ap, dummy_idx[:], 16, 16, ES)

    # ---- loads ----
    off2 = qpool.tile([16, KN * 2 * NT * 8], f32)
    nc.scalar.dma_start(off2[:], off2_ap)
    grid2 = qpool.tile([16, KN * 2 * NT * 8], f32)
    nc.scalar.dma_start(grid2[:], grid2_ap)
    offs = idxp.tile([P, NT, 2 * KN], f32)
    nc.scalar.dma_start(offs[:], off_ap.rearrange("p (t k) -> p t k", k=2 * KN))
    grid = idxp.tile([P, NT, 2 * KN], f32)
    nc.scalar.dma_start(grid[:], grid_ap.rearrange("p (t k) -> p t k", k=2 * KN))

    # ---- gather-index pipeline on partitions 0..15, split per block so the
    # first preps can launch after ~1/4 of it ----
    # layout [a, kn, coord, tau, phi]: pixel (p = phi*16+a, tau)
    co2 = qpool.tile([16, KN * 2 * NT * 8], f32)
    ci2 = qpool.tile([16, KN * 2 * NT * 8], i32)
    q32 = qpool.tile([16, KN, NT, 8], i32)
    qw16 = qpool.tile([16, KN, NT, 8], i16)
    qw = idxp.tile([P, KN, NT, 8], i16)

    nc.vector.tensor_add(co2[:], off2[:], grid2[:])
    nc.vector.tensor_scalar(co2[:], co2[:], CLIP_LO, CLIP_HI, ALU.max, ALU.min)
    nc.vector.tensor_copy(ci2[:], co2[:])
    ci2v = ci2[:].rearrange("a (k c t h) -> a k c t h", k=KN, c=2, t=NT)
    nc.vector.tensor_scalar(q32[:], ci2v[:, :, 0, :, :], 6, None,
                            ALU.arith_shift_left)
    nc.vector.tensor_tensor(qw16[:], q32[:], ci2v[:, :, 1, :, :], ALU.add)
    for g in range(8):
        eng = nc.sync if g % 2 == 0 else nc.scalar
        eng.dma_start(qw[g * 16:(g + 1) * 16, :, :, :], qw16[:])

    # ---- corner-weight pipeline (pixel px = p*32 + tau at [p, tau]) ----
    co = idxp.tile([P, NT, 2 * KN], f32)
    nc.vector.tensor_add(co[:], offs[:], grid[:])
    nc.vector.tensor_scalar(co[:], co[:], CLIP_LO, CLIP_HI, ALU.max, ALU.min)
    ci = idxp.tile([P, NT, 2 * KN], i32)
    nc.vector.tensor_copy(ci[:], co[:])            # round-to-nearest
    cf = idxp.tile([P, NT, 2 * KN], f32)
    nc.vector.tensor_copy(cf[:], ci[:])
    fr = idxp.tile([P, NT, 2 * KN], f32)
    nc.vector.tensor_sub(fr[:], co[:], cf[:])      # in [-0.5, 0.5]
    fp = idxp.tile([P, NT, 2 * KN], f32)           # frac = fr + 0.5
    nc.vector.tensor_scalar_add(fp[:], fr[:], 0.5)
    un = idxp.tile([P, NT, 2 * KN], f32)           # 1 - frac = 0.5 - fr
    nc.vector.tensor_scalar(un[:], fr[:], -1.0, 0.5, ALU.mult, ALU.add)

    fpv = fp[:].rearrange("p t (n two) -> p n t two", two=2)
    unv = un[:].rearrange("p t (n two) -> p n t two", two=2)

    # corner weights, order [00, 10, 01, 11] matching xquad layout; kn-major
    # so that (t, jp) is a contiguous merge for the combine mult
    w4 = idxp.tile([P, KN, NT, 4], bf16)
    nc.vector.tensor_tensor(w4[:, :, :, 0], unv[:, :, :, 0], unv[:, :, :, 1], ALU.mult)
    nc.vector.tensor_tensor(w4[:, :, :, 1], fpv[:, :, :, 0], unv[:, :, :, 1], ALU.mult)
    nc.vector.tensor_tensor(w4[:, :, :, 2], unv[:, :, :, 0], fpv[:, :, :, 1], ALU.mult)
    nc.vector.tensor_tensor(w4[:, :, :, 3], fpv[:, :, :, 0], fpv[:, :, :, 1], ALU.mult)



    # ---- constants ----
    ident = const.tile([P, P], f32)
    make_identity(nc, ident[:])
    ident16 = const.tile([P, P], bf16)
    nc.vector.tensor_copy(ident16[:], ident[:])
    w_sb = const.tile([P, KN, F], bf16)
    nc.sync.dma_start(w_sb[:], w_ap)  # [C, KN, F] bf16
    b_sb = const.tile([P, 1], f32)
    nc.sync.dma_start(b_sb[:], b_ap[:, None])

    # ---- main loop: one 1024-descriptor dma_gather per (kn, block). The
    # gather ucode streams descriptors to the DMA engines as it generates
    # them, so the instruction is descriptor-gen-bound (~8.7us/1024). ----
    def prep_gather(out_ap_, idx_ap_, qnum):
        nc.gpsimd.dma_gather(out_ap_, xq_ap, idx_ap_, NIDX, NIDX, ES,
                             queue_num=qnum)

    gidx = 0
    for b in range(NB):
        ops0 = ps_out.tile([P, 512], f32)
        ops1 = ps_out.tile([P, 512], f32)
        for kn in range(KN):
            G = gpool.tile([P, TB, ES], bf16)
            prep_gather(G[:], qw[:, kn, b * TB:(b + 1) * TB, :], gidx % 4)
            gidx += 1
            # xquad element order per row: (jp, c, jq); w4 j = jp*2 + jq.
            # single mult over merged (t, jp) to stay within 3 free dims
            m = mpool.tile([P, TB, 2, C, 2], bf16)
            nc.vector.tensor_tensor(
                m[:].rearrange("p t jp c jq -> p (t jp) c jq"),
                G[:].rearrange("p t (jp c jq) -> p (t jp) c jq", jp=2, c=C),
                w4[:, kn, b * TB:(b + 1) * TB, :]
                    .rearrange("p t (jp jq) -> p (t jp) jq", jp=2)
                    .unsqueeze(2).broadcast_to([P, 2 * TB, C, 2]),
                ALU.mult,
            )
            aa = abpool.tile([P, TB, C, 2], bf16)
            nc.vector.tensor_tensor(aa[:], m[:, :, 0, :, :], m[:, :, 1, :, :],
                                    ALU.add)
            dd = dpool.tile([P, TB, C], bf16)
            nc.vector.tensor_tensor(dd[:], aa[:, :, :, 0], aa[:, :, :, 1],
                                    ALU.add)

            dps = ps_dT.tile([P, TB, P], bf16)
            for tl in range(TB):
                nc.tensor.transpose(dps[:, tl, :], dd[:, tl, :], ident16[:])
            dT = dTpool.tile([P, TB, P], bf16)
            nc.scalar.copy(dT[:], dps[:])
            nc.tensor.matmul(
                ops0[:], lhsT=w_sb[:, kn, :],
                rhs=dT[:, 0:4, :].rearrange("p t c -> p (t c)"),
                start=(kn == 0), stop=(kn == KN - 1),
            )
            nc.tensor.matmul(
                ops1[:], lhsT=w_sb[:, kn, :],
                rhs=dT[:, 4:8, :].rearrange("p t c -> p (t c)"),
                start=(kn == 0), stop=(kn == KN - 1),
            )
        for half, opsx in ((0, ops0), (1, ops1)):
            oT = oTpool.tile([P, 512], f32)
            nc.scalar.activation(oT[:], opsx[:], ACT.Identity,
                                 bias=b_sb[:, 0:1], scale=1.0)
            for t4 in range(4):
                o_ps = ps_o.tile([P, P], f32)
                nc.tensor.transpose(o_ps[:], oT[:, t4 * P:(t4 + 1) * P], ident[:])
                o_sb = opool.tile([P, P], f32)
                nc.scalar.copy(o_sb[:], o_ps[:])
                pix0 = (b * TB + half * 4 + t4) * P
                nc.sync.dma_start(out_ap[pix0:pix0 + P, :], o_sb[:])


def build_nc():
    nc = bacc.Bacc(
        "TRN2",
        target_bir_lowering=False,
        debug=False,
        enable_asserts=False,
        num_devices=8,
        num_swdge_queues=4,
    )
    t_off = nc.dram_tensor("off", [P, NT * 2 * KN], f32, kind="ExternalInput")
    t_grid = nc.dram_tensor("grid", [P, NT * 2 * KN], f32, kind="ExternalInput")
    t_off2 = nc.dram_tensor("off2", [16, KN * 2 * NT * 8], f32, kind="ExternalInput")
    t_grid2 = nc.dram_tensor("grid2", [16, KN * 2 * NT * 8], f32, kind="ExternalInput")
    t_w = nc.dram_tensor("w", [C, KN, F], bf16, kind="ExternalInput")
    t_b = nc.dram_tensor("b", [F], f32, kind="ExternalInput")
    t_out = nc.dram_tensor("out", [NPIX, F], f32, kind="ExternalOutput")
    t_xq = nc.dram_tensor("xquad", [NPIX, ES], bf16, kind="ExternalInput")
    with tile.TileContext(nc) as tc:
        _body(tc, t_off, t_grid, t_off2, t_grid2, t_w, t_b, t_out, t_xq)
    nc.compile()
    return nc


def make_in_maps(x, offset, W, b):
    import ml_dtypes

    B = x.shape[0]
    # pixel relabel: pixel px = p*32 + tau lives at (partition p, tile tau)
    grid_full = _grid_full()                       # [4096, 18]
    grid_host = np.ascontiguousarray(
        grid_full.reshape(P, NT, 2 * KN) - 0.5
    ).reshape(P, NT * 2 * KN).astype(np.float32)
    # [a, kn, coord, tau, phi] <- full[(phi*16+a)*32 + tau, 2kn+coord]
    g5 = (grid_full.reshape(8, 16, NT, KN, 2).transpose(1, 3, 4, 2, 0) - 0.5)
    grid2_host = np.ascontiguousarray(g5.reshape(16, KN * 2 * NT * 8)).astype(np.float32)

    w_host = np.ascontiguousarray(
        np.asarray(W, np.float32).transpose(1, 0, 2).astype(ml_dtypes.bfloat16))
    b_host = np.ascontiguousarray(np.asarray(b, np.float32))

    in_maps = []
    for i in range(B):
        xi = np.asarray(x[i], np.float32).reshape(NPIX, C).astype(ml_dtypes.bfloat16)
        # [q, jp(x-corner), c, jq(y-corner)] = xi[q + 64*jq + jp, c]
        xq = np.zeros((NPIX, 2, C, 2), ml_dtypes.bfloat16)
        xq[:, 0, :, 0] = xi
        xq[:-64, 0, :, 1] = xi[64:]
        xq[:-1, 1, :, 0] = xi[1:]
        xq[:-65, 1, :, 1] = xi[65:]
        off_i = np.asarray(offset[i], np.float32).reshape(NPIX, 2 * KN)
        off2_i = np.ascontiguousarray(
            off_i.reshape(8, 16, NT, KN, 2).transpose(1, 3, 4, 2, 0)
            .reshape(16, KN * 2 * NT * 8)
        )
        in_maps.append(
            {
                "xquad": np.ascontiguousarray(xq.reshape(NPIX, ES)),
                "off": np.ascontiguousarray(off_i.reshape(P, NT * 2 * KN)),
                "off2": off2_i,
                "grid": grid_host,
                "grid2": grid2_host,
                "w": w_host,
                "b": b_host,
            }
        )
    return in_maps


_RESULTS_CACHE = {}


def kernel(x, offset, W, b, _trace=False):
    x = np.asarray(x)
    B = x.shape[0]
    assert x.shape == (B, H, W_IMG, C), x.shape
    nc = build_nc()
    in_maps = make_in_maps(x, offset, W, b)
    res = run_bass_kernel_spmd(nc, in_maps, core_ids=list(range(B)), trace=_trace)
    _RESULTS_CACHE["last"] = res
    out = np.stack(
        [res.results[i]["out"].reshape(NT, P, F).transpose(1, 0, 2)
         .reshape(H, W_IMG, F) for i in range(B)]
    ).astype(np.float32)
    return out
